# revision 18
# baseline (speedup 1.0000x reference)
"""Llama GQA attention (B=1, S=2048, H=4096, 32 heads / 8 KV heads, RoPE, causal)
as a tensor-parallel Bass/Tile kernel on 8 Trainium2 NeuronCores.

Sharding: core c computes Q heads [4c, 4c+4) and KV head c (GQA groups align),
full causal attention for those heads, then AllGathers the (transposed)
attention outputs and computes output features [512c, 512c+512) of o_proj.

v9 (bf16):
- Fine-grained filler weave: all proj / o_proj work is decomposed into ~1us
  "quanta" (4 k-tile matmul steps) held in per-chunk deques. The attention
  unit loop pulls one quantum after every score/PV unit, so the PE always
  has independent matmul work while the Scalar engine's EXP stream (the
  local pacer, ~1.3ns/elem) catches up. Chunk-3 attention pulls o_proj(0)/
  o_proj(1) quanta (their AllGathers complete long before), so the PE never
  idles long enough for the HAM clock gate to re-throttle.
- Softmax denominator: ONE normal-rate matmul with a ones[128,128]
  stationary computes the partition sum broadcast to all 128 partitions
  (v7 used a ones[128,1] + row-broadcast pair, each streaming at the slow
  1.2GHz narrow-output path), then reciprocal + po*rinv on DVE.
- Critical input DMAs are emitted before any const-pool memset so the DMA
  queues start moving at ~6us instead of ~11.6us (gpsimd preamble work was
  gating the first transfers).
- Proj groups are single-PSUM-bank ([128,512]); chunk-0 runs V,K,q0 up
  front across three pools, then starts attention with q1..q3 as fillers
  (pull 4/unit), with emission-order guards ensuring each head's rope is
  emitted before its scores (no forward deps exist for stale reads).
- PSUM: scores 2x[128,2,512] (also chunk-0 proj + tail o_proj, for double
  buffering), filler proj/o_proj 1x[128,512], denominator/V-transpose
  1x bank, po 2x[128,512] = exactly 8 banks.
- Finishers ride 2 units into the next head's stream; the AllGather for a
  chunk triggers right after its last att DMA (gpsimd carries only AG
  triggers); exp batched 2 k-blocks per ACT op; DEPTH=3 score pipeline.
"""
import numpy as np
import ml_dtypes
from collections import deque
from contextlib import ExitStack

import concourse.bass as bass
import concourse.mybir as mybir
import concourse.tile as tile
from concourse import bacc
from concourse.bass import ts, ds
from concourse.masks import make_identity

N_CORES = 8
S = 2048
HIDDEN = 4096
NUM_HEADS = 32
HEAD_DIM = 128
HEADS_PER_CORE = NUM_HEADS // N_CORES          # 4
QSLICE = HEADS_PER_CORE * HEAD_DIM             # 512
KT = HIDDEN // 128                             # 32 contraction tiles
SC = S // 512                                  # 4 seq chunks of 512
ROPE_THETA = 10000.0

F32 = mybir.dt.float32
BF16 = mybir.dt.bfloat16

_cache = {}


def build_nc():
    nc = bacc.Bacc("TRN2", target_bir_lowering=False, debug=False,
                   num_devices=N_CORES)
    xT = nc.dram_tensor("xT", [HIDDEN, S], BF16, kind="ExternalInput").ap()
    wqT = nc.dram_tensor("wqT", [HIDDEN, QSLICE], BF16, kind="ExternalInput").ap()
    wkvT = nc.dram_tensor("wkvT", [HIDDEN, 2 * HEAD_DIM], BF16,
                          kind="ExternalInput").ap()
    woT = nc.dram_tensor("woT", [HIDDEN, QSLICE], BF16, kind="ExternalInput").ap()
    cosT = nc.dram_tensor("cosT", [HEAD_DIM, S], F32, kind="ExternalInput").ap()
    sinT = nc.dram_tensor("sinT", [HEAD_DIM, S], F32, kind="ExternalInput").ap()
    outT = nc.dram_tensor("outT", [QSLICE, S], F32, kind="ExternalOutput").ap()

    xT_r = xT.rearrange("(kt p) s -> p kt s", p=128)
    wqT_r = wqT.rearrange("(kt p) m -> p kt m", p=128)
    wkvT_r = wkvT.rearrange("(kt p) m -> p kt m", p=128)
    woT_r = woT.rearrange("(kt p) m -> p kt m", p=128)

    with tile.TileContext(nc) as tc, ExitStack() as ctx:
        const = ctx.enter_context(tc.tile_pool(name="const", bufs=1))
        bigw = ctx.enter_context(tc.tile_pool(name="bigw", bufs=1))
        slab = ctx.enter_context(tc.tile_pool(name="slab", bufs=2))
        f32t = ctx.enter_context(tc.tile_pool(name="f32t", bufs=4))
        ppool = ctx.enter_context(tc.tile_pool(name="ppool", bufs=5))
        accp = ctx.enter_context(tc.tile_pool(name="accp", bufs=2))
        smalls = ctx.enter_context(tc.tile_pool(name="smalls", bufs=2))
        rinvp = ctx.enter_context(tc.tile_pool(name="rinvp", bufs=2))
        otp = ctx.enter_context(tc.tile_pool(name="otp", bufs=4))
        dram = ctx.enter_context(tc.tile_pool(name="dram", bufs=1, space="DRAM"))
        # PSUM banks: psb 2x[128,2,512]=4, ppj 1x[128,512]=1,
        #             pden 1x(den|ptr)=1, ppo 2x[128,512]=2  -> 8 total
        psb = ctx.enter_context(tc.tile_pool(name="psb", bufs=2, space="PSUM"))
        ppj = ctx.enter_context(tc.tile_pool(name="ppj", bufs=1, space="PSUM"))
        pden = ctx.enter_context(tc.tile_pool(name="pden", bufs=1, space="PSUM"))
        ppo = ctx.enter_context(tc.tile_pool(name="ppo", bufs=2, space="PSUM"))

        # ---- persistent tiles (allocation only; no instructions yet)
        ones2 = const.tile([128, 128], BF16, name="ones2")
        ident = const.tile([128, 128], BF16, name="ident")
        maskb = const.tile([128, 4, 512], BF16, name="maskb")
        cos_sb = const.tile([128, S], F32, name="cos_sb")
        sin_sb = const.tile([128, S], F32, name="sin_sb")
        qT_sb = const.tile([128, HEADS_PER_CORE, S], BF16, name="qT_sb")
        kT_sb = const.tile([128, S], BF16, name="kT_sb")
        v_sb = const.tile([128, S // 128, HEAD_DIM], BF16, name="v_sb")
        wkv_sb = const.tile([128, KT, 2 * HEAD_DIM], BF16, name="wkv_sb")
        # wq and wo share one 32KB/part slot; wo loads once proj is done
        wq_sb = bigw.tile([128, KT, QSLICE], BF16, tag="bigw", name="wq_sb")

        ag_ins = [dram.tile([QSLICE, 512], BF16, tag=f"agin{j}",
                            name=f"agin{j}") for j in range(SC)]
        ag_outs = [dram.tile([NUM_HEADS * HEAD_DIM, 512], BF16,
                             addr_space="Shared", tag=f"agout{j}",
                             name=f"agout{j}") for j in range(SC)]

        # ---- critical DMAs FIRST (before any const memset, so the first
        # transfers aren't queued behind gpsimd preamble work)
        x_slab0 = slab.tile([128, KT, 512], BF16, tag="slab", name="x_slab0")
        for g in range(4):  # fine-grained first quarter for a fast ramp
            kts = ds(g, 1)
            nc.sync.dma_start(x_slab0[:, kts, :], xT_r[:, kts, 0:512])
            nc.sync.dma_start(wkv_sb[:, kts, :], wkvT_r[:, kts, :])
        for g in range(1, 8):
            kts = ds(4 * g, 4)
            nc.sync.dma_start(x_slab0[:, kts, :], xT_r[:, kts, 0:512])
            nc.sync.dma_start(wkv_sb[:, kts, :], wkvT_r[:, kts, :])
        nc.sync.dma_start(cos_sb[:], cosT[:])
        nc.sync.dma_start(sin_sb[:], sinT[:])
        for g in range(8):
            kts = ds(4 * g, 4)
            nc.sync.dma_start(wq_sb[:, kts, :], wqT_r[:, kts, :])

        # ---- constants
        nc.vector.memset(ones2[:], 1.0)
        make_identity(nc, ident[:])
        # causal mask bias tiles: maskb[k,d,q] = 0 where q-128d-k >= 0 else -1e9
        nc.gpsimd.memset(maskb[:], 0.0)
        for d in range(4):
            nc.gpsimd.affine_select(
                maskb[:, d, :], maskb[:, d, :], pattern=[[1, 512]],
                compare_op=mybir.AluOpType.is_ge, fill=-1.0e9,
                base=-128 * d, channel_multiplier=-1)

        def rope(dst, src, s):
            """dst = src*cos + rotate_half(src)*sin_signed for seq chunk s."""
            rot = f32t.tile([128, 512], F32, tag="f32t", name="rot")
            nc.vector.tensor_tensor(rot[0:64, :], src[64:128, :],
                                    sin_sb[0:64, ts(s, 512)],
                                    mybir.AluOpType.mult)
            nc.vector.tensor_tensor(rot[64:128, :], src[0:64, :],
                                    sin_sb[64:128, ts(s, 512)],
                                    mybir.AluOpType.mult)
            cq = f32t.tile([128, 512], F32, tag="f32t", name="cq")
            nc.vector.tensor_tensor(cq[:], src[:], cos_sb[:, ts(s, 512)],
                                    mybir.AluOpType.mult)
            nc.vector.tensor_tensor(dst, cq[:], rot[:], mybir.AluOpType.add)

        x_slabs = {0: x_slab0}
        wo_holder = {}
        a_slabs = {}
        ready = {}  # emission-order guards: set when a proj epilogue emits

        # ================= filler quantum machinery =================
        # Each quantum is a zero-arg closure emitting ~1us of PE work (plus
        # any non-PE epilogue). Finishers are queued with a tick delay so
        # their producer chains (DVE acc) settle inside the next stream.
        fins = []                                   # [delay_ticks, fn]
        phase_q = [deque() for _ in range(SC + 1)]  # per-chunk filler deques

        def pull_filler(s):
            for d in phase_q:
                if d:
                    d.popleft()()
                    return True
            return False

        def tick(s, pull=True):
            due = []
            for item in fins:
                item[0] -= 1
            while fins and fins[0][0] <= 0:
                due.append(fins.pop(0)[1])
            for f in due:
                f()
            if not due and pull:
                return pull_filler(s)
            return True

        # ---- single-bank proj group generators (32 MMs + epilogue each)
        def gen_group(pool, mm_of_kt, epi):
            box = {}

            def begin():
                box["p"] = pool.tile([128, 512], F32, tag="big", name="pacc")
                step(0)()

            def step(k0):
                def run():
                    for kt in range(k0, k0 + 4):
                        nc.tensor.matmul(box["p"][:], *mm_of_kt(kt),
                                         start=(kt == 0), stop=(kt == KT - 1))
                return run

            def end():
                epi(box["p"])

            return [begin] + [step(k) for k in range(4, KT, 4)] + [end]

        def gen_proj_v(s, pool):
            def epi(p):
                vt = smalls.tile([128, 512], BF16, tag="vt", name="vt")
                nc.scalar.copy(vt[:], p[:])
                ptr = pden.tile([128, 4, 128], BF16, tag="big", name="ptr")
                for t in range(4):
                    nc.tensor.transpose(ptr[:, t, :], vt[:, ts(t, 128)],
                                        ident[:])
                nc.scalar.copy(v_sb[:, ds(4 * s, 4), :], ptr[:])
                ready[("v", s)] = True
            return gen_group(
                pool,
                lambda kt: (wkv_sb[:, kt, ds(128, 128)], x_slabs[s][:, kt, :]),
                epi)

        def gen_proj_k(s, pool):
            # rope reads the PSUM accumulator directly (cross-partition
            # tensor_tensor needs a PSUM input; SBUF+SBUF must align)
            def epi(p):
                rope(kT_sb[:, ts(s, 512)], p[:], s)
                ready[("k", s)] = True
            return gen_group(
                pool,
                lambda kt: (wkv_sb[:, kt, ds(0, 128)], x_slabs[s][:, kt, :]),
                epi)

        def gen_proj_q(s, h, pool):
            def epi(p):
                rope(qT_sb[:, h, ts(s, 512)], p[:], s)
                ready[("q", s, h)] = True
            return gen_group(
                pool,
                lambda kt: (wq_sb[:, kt, ts(h, 128)], x_slabs[s][:, kt, :]),
                epi)

        def gen_oproj(s, ft, pool):
            def epi(p):
                ot = otp.tile([128, 512], F32, tag="ot", name="ot")
                nc.scalar.copy(ot[:], p[:])
                nc.sync.dma_start(outT[ds(ft * 128, 128), ts(s, 512)], ot[:])
            return gen_group(
                pool,
                lambda kt: (wo_holder[0][:, kt, ts(ft, 128)],
                            a_slabs[s][:, kt, :]),
                epi)

        def load_x(s):
            def run():
                xs = slab.tile([128, KT, 512], BF16, tag="slab",
                               name="x_slab")
                nc.sync.dma_start(xs[:], xT_r[:, :, ts(s, 512)])
                x_slabs[s] = xs
            return run

        def load_wo():
            def run():
                wo = bigw.tile([128, KT, QSLICE], BF16, tag="bigw",
                               name="wo_sb")
                for f in range(4):
                    nc.sync.dma_start(wo[:, :, ts(f, 128)],
                                      woT_r[:, :, ts(f, 128)])
                wo_holder[0] = wo
            return run

        def load_a(s):
            def run():
                a_slab = slab.tile([128, KT, 512], BF16, tag="slab",
                                   name="a_slab")
                ag_r = ag_outs[s].rearrange("(kt p) s -> p kt s", p=128)
                for g in range(4):
                    kts = ds(8 * g, 8)
                    nc.sync.dma_start(a_slab[:, kts, :], ag_r[:, kts, :])
                a_slabs[s] = a_slab
            return run

        # ================= attention =================
        def attn_head(j, h, pull_num, pull_den):
            """Scores + exp + mask + denom-acc + PV, DEPTH=3 pipeline.
            Pulls pull_num filler quanta every pull_den units."""
            nunit = 2 * (j + 1)
            po = ppo.tile([128, 512], F32, tag="po", name="po")
            acc = accp.tile([128, 512], BF16, tag="acc", name="acc")
            nc.vector.memset(acc[:], 0.0)
            # diagonal (masked) units first; their scores/exp/PV shrink to
            # the causally-valid rectangle q >= 128d
            order = list(range(nunit))[::-1]

            def emit_scores(idx):
                u = order[idx]
                ps = psb.tile([128, 2, 512], F32, tag="big", name="ps")
                halves = []
                for w in range(2):
                    ki = 2 * u + w
                    d = ki - 4 * j
                    w0 = 128 * d if d > 0 else 0
                    halves.append(w0)
                    nc.tensor.matmul(ps[:, w, w0:512],
                                     kT_sb[:, ts(ki, 128)],
                                     qT_sb[:, h, ds(512 * j + w0, 512 - w0)],
                                     start=True, stop=(d < 0))
                    if d >= 0:  # diagonal block: add -1e9 causal bias on PE
                        nc.tensor.matmul(ps[:, w, w0:512], ident[:],
                                         maskb[:, d, w0:512],
                                         start=False, stop=True)
                pT = ppool.tile([128, 2, 512], BF16, tag="pT", name="pT")
                if halves[0] == 0 and halves[1] == 0:
                    nc.scalar.activation(pT[:], ps[:],
                                         mybir.ActivationFunctionType.Exp)
                else:
                    for w in range(2):
                        w0 = halves[w]
                        nc.scalar.activation(pT[:, w, w0:512],
                                             ps[:, w, w0:512],
                                             mybir.ActivationFunctionType.Exp)
                for w in range(2):
                    w0 = halves[w]
                    nc.vector.tensor_tensor(acc[:, w0:512], acc[:, w0:512],
                                            pT[:, w, w0:512],
                                            mybir.AluOpType.add)
                return pT, halves

            DEPTH = 4
            pts = [emit_scores(ii) for ii in range(min(DEPTH, nunit))]
            credit = 0
            for idx in range(nunit):
                pT, halves = pts[idx]
                if idx + DEPTH < nunit:
                    pts.append(emit_scores(idx + DEPTH))
                # fillers ride between the scores emission and the PV that
                # needs this unit's exp, maximizing the ACT slack
                credit += pull_num
                while credit >= pull_den:
                    tick(j, pull=True)
                    credit -= pull_den
                if credit > 0 and pull_num < pull_den:
                    tick(j, pull=False)
                u = order[idx]
                for w in range(2):
                    ki = 2 * u + w
                    w0 = halves[w]
                    nc.tensor.matmul(po[:, w0:512], v_sb[:, ki, :],
                                     pT[:, w, w0:512],
                                     start=(idx == 0 and w == 0),
                                     stop=(idx == nunit - 1 and w == 1))
            return po, acc

        def attn_fin(j, h, st, last):
            po, acc = st

            def run():
                # one normal-rate matmul: den[p,q] = sum_k acc[k,q] for all
                # p (the ones stationary broadcasts the partition sum)
                den = pden.tile([128, 512], F32, tag="big", name="den")
                nc.tensor.matmul(den[:], ones2[:], acc[:], start=True,
                                 stop=True)
                rinv = rinvp.tile([128, 512], F32, tag="rinv", name="rinv")
                nc.vector.reciprocal_approx_fast(rinv[:], den[:])
                att = smalls.tile([128, 512], BF16, tag="att", name="att")
                nc.vector.tensor_tensor(att[:], po[:], rinv[:],
                                        mybir.AluOpType.mult)
                nc.sync.dma_start(ag_ins[j][ts(h, 128), :], att[:])
                if last:
                    nc.gpsimd.collective_compute(
                        "AllGather", mybir.AluOpType.bypass,
                        replica_groups=[list(range(N_CORES))],
                        ins=[ag_ins[j].opt()], outs=[ag_outs[j].opt()],
                    )
            return run

        # ================= schedule =================
        # filler phases: chunk-s attention pulls proj(s+1); chunk-3 pulls
        # o_proj(0)/(1); the tail drains o_proj(2)/(3).
        for s in (1, 2, 3):
            phase_q[s - 1].append(load_x(s))
            phase_q[s - 1].extend(gen_proj_v(s, ppj))
            phase_q[s - 1].extend(gen_proj_k(s, ppj))
            for h in range(HEADS_PER_CORE):
                phase_q[s - 1].extend(gen_proj_q(s, h, ppj))
        phase_q[2].append(load_wo())
        for s in (0, 1):
            phase_q[3].append(load_a(s))
            for ft in range(4):
                phase_q[3].extend(gen_oproj(s, ft, ppj))
        for s in (2, 3):
            phase_q[4].append(load_a(s))
            for ft in range(4):
                phase_q[4].extend(gen_oproj(s, ft, psb))

        # chunk-0: emit only V, K, q0 up front (spread over three PSUM
        # pools so the rope chains never serialize the accumulator bank);
        # q1..q3 ride as fillers inside heads 0-2 (pull 4/unit covers one
        # q group per head).
        for q in gen_proj_v(0, psb):
            q()
        for q in gen_proj_k(0, psb):
            q()
        for q in gen_proj_q(0, 0, ppj):
            q()
        c0 = deque()
        c0.extend(gen_proj_q(0, 1, pden))
        c0.extend(gen_proj_q(0, 2, psb))
        c0.extend(gen_proj_q(0, 3, psb))
        c0.extend(phase_q[0])
        phase_q[0] = c0
        tick(0)
        tick(0)

        # per-chunk (pull_num, pull_den): c0 covers its own q groups, c1/c2
        # absorb most of the next chunk's proj so the drain shrinks, c3
        # paces o_proj; the last head minimizes pulls so the final
        # AllGather triggers ASAP.
        rates = {0: (4, 1), 1: (2, 1), 2: (2, 1), 3: (1, 1)}
        for s in range(SC):
            for h in range(HEADS_PER_CORE):
                # emission-order guard: this head's q rope (and the chunk's
                # k/v epilogues) must be EMITTED before its scores/PV are,
                # else they'd read stale SBUF (no forward deps exist)
                while not (ready.get(("q", s, h)) and ready.get(("k", s))
                           and ready.get(("v", s))):
                    if not tick(s):
                        raise RuntimeError(f"filler underflow at {s},{h}")
                if s == SC - 1 and h == HEADS_PER_CORE - 1:
                    pn, pd = 1, 3
                else:
                    pn, pd = rates[s]
                st = attn_head(s, h, pn, pd)
                fins.append([2, attn_fin(s, h, st,
                                         last=(h == HEADS_PER_CORE - 1))])
            if s < SC - 1:
                # drain most of this chunk's proj fillers (next chunk's
                # scores need qT/kT/v soon) but keep a tail of quanta to
                # absorb the rope-boundary stalls inside the next heads
                while len(phase_q[s]) > 12:
                    tick(s)

        # ---- tail: flush remaining fins + o_proj quanta
        while fins or any(phase_q):
            if not any(phase_q):
                for item in fins:
                    item[0] = 0
            tick(SC - 1)

    nc.finalize()
    return nc


def _prep_inputs(hidden_states, Wq, Wk, Wv, Wo, position_ids):
    """Slice/cast per-core inputs (host-side layout prep only)."""
    bf = ml_dtypes.bfloat16
    x = np.ascontiguousarray(np.asarray(hidden_states, np.float32)[0].T).astype(bf)
    scale = 1.0 / np.sqrt(HEAD_DIM)
    # rotary tables, [head_dim, seq]; sin signed (first half negated)
    invf_half = (1.0 / (ROPE_THETA ** (np.arange(0, HEAD_DIM, 2, dtype=np.float64)
                                       / HEAD_DIM)))
    invf = np.concatenate([invf_half, invf_half])  # [128]
    pos = np.asarray(position_ids, np.float64).reshape(S)
    ang = invf[:, None] * pos[None, :]             # [128, S]
    cosT = np.cos(ang).astype(np.float32)
    sinT = np.sin(ang).astype(np.float32)
    sinT[:HEAD_DIM // 2] *= -1.0
    in_maps = []
    for c in range(N_CORES):
        wq_c = (np.asarray(Wq, np.float32)[c * QSLICE:(c + 1) * QSLICE] * scale)
        wk_c = np.asarray(Wk, np.float32)[c * HEAD_DIM:(c + 1) * HEAD_DIM]
        wv_c = np.asarray(Wv, np.float32)[c * HEAD_DIM:(c + 1) * HEAD_DIM]
        wkv_c = np.concatenate([wk_c, wv_c], axis=0)   # [256, 4096]
        wo_c = np.asarray(Wo, np.float32)[c * QSLICE:(c + 1) * QSLICE]
        in_maps.append({
            "xT": x,
            "wqT": np.ascontiguousarray(wq_c.T).astype(bf),
            "wkvT": np.ascontiguousarray(wkv_c.T).astype(bf),
            "woT": np.ascontiguousarray(wo_c.T).astype(bf),
            "cosT": cosT,
            "sinT": sinT,
        })
    return in_maps


def kernel(hidden_states, Wq, Wk, Wv, Wo, position_ids):
    from concourse.bass_utils import run_bass_kernel_spmd
    if "nc" not in _cache:
        _cache["nc"] = build_nc()
    nc = _cache["nc"]
    in_maps = _prep_inputs(hidden_states, Wq, Wk, Wv, Wo, position_ids)
    res = run_bass_kernel_spmd(nc, in_maps, core_ids=list(range(N_CORES)))
    out = np.concatenate([res.results[c]["outT"].T for c in range(N_CORES)], axis=1)
    return out[None].astype(np.float32)


# revision 19
# speedup vs baseline: 1.0094x; 1.0094x over previous
"""Llama GQA attention (B=1, S=2048, H=4096, 32 heads / 8 KV heads, RoPE, causal)
as a tensor-parallel Bass/Tile kernel on 8 Trainium2 NeuronCores.

Sharding: core c computes Q heads [4c, 4c+4) and KV head c (GQA groups align),
full causal attention for those heads, then AllGathers the (transposed)
attention outputs and computes output features [512c, 512c+512) of o_proj.

v9 (bf16):
- Fine-grained filler weave: all proj / o_proj work is decomposed into ~1us
  "quanta" (4 k-tile matmul steps) held in per-chunk deques. The attention
  unit loop pulls one quantum after every score/PV unit, so the PE always
  has independent matmul work while the Scalar engine's EXP stream (the
  local pacer, ~1.3ns/elem) catches up. Chunk-3 attention pulls o_proj(0)/
  o_proj(1) quanta (their AllGathers complete long before), so the PE never
  idles long enough for the HAM clock gate to re-throttle.
- Softmax denominator: ONE normal-rate matmul with a ones[128,128]
  stationary computes the partition sum broadcast to all 128 partitions
  (v7 used a ones[128,1] + row-broadcast pair, each streaming at the slow
  1.2GHz narrow-output path), then reciprocal + po*rinv on DVE.
- Critical input DMAs are emitted before any const-pool memset so the DMA
  queues start moving at ~6us instead of ~11.6us (gpsimd preamble work was
  gating the first transfers).
- Proj groups are single-PSUM-bank ([128,512]); chunk-0 runs V,K,q0 up
  front across three pools, then starts attention with q1..q3 as fillers
  (pull 4/unit), with emission-order guards ensuring each head's rope is
  emitted before its scores (no forward deps exist for stale reads).
- PSUM: scores 2x[128,2,512] (also chunk-0 proj + tail o_proj, for double
  buffering), filler proj/o_proj 1x[128,512], denominator/V-transpose
  1x bank, po 2x[128,512] = exactly 8 banks.
- Finishers ride 2 units into the next head's stream; the AllGather for a
  chunk triggers right after its last att DMA (gpsimd carries only AG
  triggers); exp batched 2 k-blocks per ACT op; DEPTH=3 score pipeline.
"""
import numpy as np
import ml_dtypes
from collections import deque
from contextlib import ExitStack

import concourse.bass as bass
import concourse.mybir as mybir
import concourse.tile as tile
from concourse import bacc
from concourse.bass import ts, ds
from concourse.masks import make_identity

N_CORES = 8
S = 2048
HIDDEN = 4096
NUM_HEADS = 32
HEAD_DIM = 128
HEADS_PER_CORE = NUM_HEADS // N_CORES          # 4
QSLICE = HEADS_PER_CORE * HEAD_DIM             # 512
KT = HIDDEN // 128                             # 32 contraction tiles
SC = S // 512                                  # 4 seq chunks of 512
ROPE_THETA = 10000.0

F32 = mybir.dt.float32
BF16 = mybir.dt.bfloat16

_cache = {}


def build_nc():
    nc = bacc.Bacc("TRN2", target_bir_lowering=False, debug=False,
                   num_devices=N_CORES)
    xT = nc.dram_tensor("xT", [HIDDEN, S], BF16, kind="ExternalInput").ap()
    wqT = nc.dram_tensor("wqT", [HIDDEN, QSLICE], BF16, kind="ExternalInput").ap()
    wkvT = nc.dram_tensor("wkvT", [HIDDEN, 2 * HEAD_DIM], BF16,
                          kind="ExternalInput").ap()
    woT = nc.dram_tensor("woT", [HIDDEN, QSLICE], BF16, kind="ExternalInput").ap()
    cosT = nc.dram_tensor("cosT", [HEAD_DIM, S], F32, kind="ExternalInput").ap()
    sinT = nc.dram_tensor("sinT", [HEAD_DIM, S], F32, kind="ExternalInput").ap()
    outT = nc.dram_tensor("outT", [QSLICE, S], F32, kind="ExternalOutput").ap()

    xT_r = xT.rearrange("(kt p) s -> p kt s", p=128)
    wqT_r = wqT.rearrange("(kt p) m -> p kt m", p=128)
    wkvT_r = wkvT.rearrange("(kt p) m -> p kt m", p=128)
    woT_r = woT.rearrange("(kt p) m -> p kt m", p=128)

    with tile.TileContext(nc) as tc, ExitStack() as ctx:
        const = ctx.enter_context(tc.tile_pool(name="const", bufs=1))
        bigw = ctx.enter_context(tc.tile_pool(name="bigw", bufs=1))
        slab = ctx.enter_context(tc.tile_pool(name="slab", bufs=2))
        f32t = ctx.enter_context(tc.tile_pool(name="f32t", bufs=4))
        ppool = ctx.enter_context(tc.tile_pool(name="ppool", bufs=5))
        accp = ctx.enter_context(tc.tile_pool(name="accp", bufs=2))
        smalls = ctx.enter_context(tc.tile_pool(name="smalls", bufs=2))
        rinvp = ctx.enter_context(tc.tile_pool(name="rinvp", bufs=2))
        otp = ctx.enter_context(tc.tile_pool(name="otp", bufs=4))
        dram = ctx.enter_context(tc.tile_pool(name="dram", bufs=1, space="DRAM"))
        # PSUM banks: psb 2x[128,2,512]=4, ppj 1x[128,512]=1,
        #             pden 1x(den|ptr)=1, ppo 2x[128,512]=2  -> 8 total
        psb = ctx.enter_context(tc.tile_pool(name="psb", bufs=2, space="PSUM"))
        ppj = ctx.enter_context(tc.tile_pool(name="ppj", bufs=1, space="PSUM"))
        pden = ctx.enter_context(tc.tile_pool(name="pden", bufs=1, space="PSUM"))
        ppo = ctx.enter_context(tc.tile_pool(name="ppo", bufs=2, space="PSUM"))

        # ---- persistent tiles (allocation only; no instructions yet)
        ones2 = const.tile([128, 128], BF16, name="ones2")
        ident = const.tile([128, 128], BF16, name="ident")
        maskb = const.tile([128, 4, 512], BF16, name="maskb")
        cos_sb = const.tile([128, S], F32, name="cos_sb")
        sin_sb = const.tile([128, S], F32, name="sin_sb")
        qT_sb = const.tile([128, HEADS_PER_CORE, S], BF16, name="qT_sb")
        kT_sb = const.tile([128, S], BF16, name="kT_sb")
        v_sb = const.tile([128, S // 128, HEAD_DIM], BF16, name="v_sb")
        wkv_sb = const.tile([128, KT, 2 * HEAD_DIM], BF16, name="wkv_sb")
        # wq and wo share one 32KB/part slot; wo loads once proj is done
        wq_sb = bigw.tile([128, KT, QSLICE], BF16, tag="bigw", name="wq_sb")

        ag_ins = [dram.tile([QSLICE, 512], BF16, tag=f"agin{j}",
                            name=f"agin{j}") for j in range(SC)]
        ag_outs = [dram.tile([NUM_HEADS * HEAD_DIM, 512], BF16,
                             addr_space="Shared", tag=f"agout{j}",
                             name=f"agout{j}") for j in range(SC)]

        # ---- critical DMAs FIRST (before any const memset, so the first
        # transfers aren't queued behind gpsimd preamble work)
        x_slab0 = slab.tile([128, KT, 512], BF16, tag="slab", name="x_slab0")
        for g in range(4):  # fine-grained first quarter for a fast ramp
            kts = ds(g, 1)
            nc.sync.dma_start(x_slab0[:, kts, :], xT_r[:, kts, 0:512])
            nc.sync.dma_start(wkv_sb[:, kts, :], wkvT_r[:, kts, :])
        for g in range(1, 8):
            kts = ds(4 * g, 4)
            nc.sync.dma_start(x_slab0[:, kts, :], xT_r[:, kts, 0:512])
            nc.sync.dma_start(wkv_sb[:, kts, :], wkvT_r[:, kts, :])
        nc.sync.dma_start(cos_sb[:], cosT[:])
        nc.sync.dma_start(sin_sb[:], sinT[:])
        for g in range(8):
            kts = ds(4 * g, 4)
            nc.sync.dma_start(wq_sb[:, kts, :], wqT_r[:, kts, :])

        # ---- constants
        nc.vector.memset(ones2[:], 1.0)
        make_identity(nc, ident[:])
        # causal mask bias tiles: maskb[k,d,q] = 0 where q-128d-k >= 0 else -1e9
        nc.gpsimd.memset(maskb[:], 0.0)
        for d in range(4):
            nc.gpsimd.affine_select(
                maskb[:, d, :], maskb[:, d, :], pattern=[[1, 512]],
                compare_op=mybir.AluOpType.is_ge, fill=-1.0e9,
                base=-128 * d, channel_multiplier=-1)

        def rope(dst, src, s):
            """dst = src*cos + rotate_half(src)*sin_signed for seq chunk s."""
            rot = f32t.tile([128, 512], F32, tag="f32t", name="rot")
            nc.vector.tensor_tensor(rot[0:64, :], src[64:128, :],
                                    sin_sb[0:64, ts(s, 512)],
                                    mybir.AluOpType.mult)
            nc.vector.tensor_tensor(rot[64:128, :], src[0:64, :],
                                    sin_sb[64:128, ts(s, 512)],
                                    mybir.AluOpType.mult)
            cq = f32t.tile([128, 512], F32, tag="f32t", name="cq")
            nc.vector.tensor_tensor(cq[:], src[:], cos_sb[:, ts(s, 512)],
                                    mybir.AluOpType.mult)
            nc.vector.tensor_tensor(dst, cq[:], rot[:], mybir.AluOpType.add)

        x_slabs = {0: x_slab0}
        wo_holder = {}
        a_slabs = {}
        ready = {}  # emission-order guards: set when a proj epilogue emits

        # ================= filler quantum machinery =================
        # Each quantum is a zero-arg closure emitting ~1us of PE work (plus
        # any non-PE epilogue). Finishers are queued with a tick delay so
        # their producer chains (DVE acc) settle inside the next stream.
        fins = []                                   # [delay_ticks, fn]
        phase_q = [deque() for _ in range(SC + 1)]  # per-chunk filler deques

        def pull_filler(s):
            for d in phase_q:
                if d:
                    d.popleft()()
                    return True
            return False

        def tick(s, pull=True):
            due = []
            for item in fins:
                item[0] -= 1
            while fins and fins[0][0] <= 0:
                due.append(fins.pop(0)[1])
            for f in due:
                f()
            if not due and pull:
                return pull_filler(s)
            return True

        # ---- single-bank proj group generators (32 MMs + epilogue each)
        def gen_group(pool, mm_of_kt, epi):
            box = {}

            def begin():
                box["p"] = pool.tile([128, 512], F32, tag="big", name="pacc")
                step(0)()

            def step(k0):
                def run():
                    for kt in range(k0, k0 + 4):
                        nc.tensor.matmul(box["p"][:], *mm_of_kt(kt),
                                         start=(kt == 0), stop=(kt == KT - 1))
                return run

            def end():
                epi(box["p"])

            return [begin] + [step(k) for k in range(4, KT, 4)] + [end]

        def gen_proj_v(s, pool):
            def epi(p):
                vt = smalls.tile([128, 512], BF16, tag="vt", name="vt")
                nc.scalar.copy(vt[:], p[:])
                ptr = pden.tile([128, 4, 128], BF16, tag="big", name="ptr")
                for t in range(4):
                    nc.tensor.transpose(ptr[:, t, :], vt[:, ts(t, 128)],
                                        ident[:])
                nc.scalar.copy(v_sb[:, ds(4 * s, 4), :], ptr[:])
                ready[("v", s)] = True
            return gen_group(
                pool,
                lambda kt: (wkv_sb[:, kt, ds(128, 128)], x_slabs[s][:, kt, :]),
                epi)

        def gen_proj_k(s, pool):
            # rope reads the PSUM accumulator directly (cross-partition
            # tensor_tensor needs a PSUM input; SBUF+SBUF must align)
            def epi(p):
                rope(kT_sb[:, ts(s, 512)], p[:], s)
                ready[("k", s)] = True
            return gen_group(
                pool,
                lambda kt: (wkv_sb[:, kt, ds(0, 128)], x_slabs[s][:, kt, :]),
                epi)

        def gen_proj_q(s, h, pool):
            def epi(p):
                rope(qT_sb[:, h, ts(s, 512)], p[:], s)
                ready[("q", s, h)] = True
            return gen_group(
                pool,
                lambda kt: (wq_sb[:, kt, ts(h, 128)], x_slabs[s][:, kt, :]),
                epi)

        def gen_oproj(s, ft, pool):
            def epi(p):
                ot = otp.tile([128, 512], F32, tag="ot", name="ot")
                nc.scalar.copy(ot[:], p[:])
                nc.sync.dma_start(outT[ds(ft * 128, 128), ts(s, 512)], ot[:])
            return gen_group(
                pool,
                lambda kt: (wo_holder[0][:, kt, ts(ft, 128)],
                            a_slabs[s][:, kt, :]),
                epi)

        def load_x(s):
            def run():
                xs = slab.tile([128, KT, 512], BF16, tag="slab",
                               name="x_slab")
                nc.sync.dma_start(xs[:], xT_r[:, :, ts(s, 512)])
                x_slabs[s] = xs
            return run

        def load_wo():
            def run():
                wo = bigw.tile([128, KT, QSLICE], BF16, tag="bigw",
                               name="wo_sb")
                for f in range(4):
                    nc.sync.dma_start(wo[:, :, ts(f, 128)],
                                      woT_r[:, :, ts(f, 128)])
                wo_holder[0] = wo
            return run

        def load_a(s):
            def run():
                a_slab = slab.tile([128, KT, 512], BF16, tag="slab",
                                   name="a_slab")
                ag_r = ag_outs[s].rearrange("(kt p) s -> p kt s", p=128)
                for g in range(4):
                    kts = ds(8 * g, 8)
                    nc.sync.dma_start(a_slab[:, kts, :], ag_r[:, kts, :])
                a_slabs[s] = a_slab
            return run

        # ================= attention =================
        def attn_head(j, h, pull_num, pull_den):
            """Scores + exp + mask + denom-acc + PV, DEPTH=3 pipeline.
            Pulls pull_num filler quanta every pull_den units."""
            nunit = 2 * (j + 1)
            po = ppo.tile([128, 512], F32, tag="po", name="po")
            acc = accp.tile([128, 512], BF16, tag="acc", name="acc")
            nc.vector.memset(acc[:], 0.0)
            # diagonal (masked) units first; their scores/exp/PV shrink to
            # the causally-valid rectangle q >= 128d
            order = list(range(nunit))[::-1]

            def emit_scores(idx):
                u = order[idx]
                ps = psb.tile([128, 2, 512], F32, tag="big", name="ps")
                halves = []
                for w in range(2):
                    ki = 2 * u + w
                    d = ki - 4 * j
                    w0 = 128 * d if d > 0 else 0
                    halves.append(w0)
                    nc.tensor.matmul(ps[:, w, w0:512],
                                     kT_sb[:, ts(ki, 128)],
                                     qT_sb[:, h, ds(512 * j + w0, 512 - w0)],
                                     start=True, stop=(d < 0))
                    if d >= 0:  # diagonal block: add -1e9 causal bias on PE
                        nc.tensor.matmul(ps[:, w, w0:512], ident[:],
                                         maskb[:, d, w0:512],
                                         start=False, stop=True)
                pT = ppool.tile([128, 2, 512], BF16, tag="pT", name="pT")
                if halves[0] == 0 and halves[1] == 0:
                    nc.scalar.activation(pT[:], ps[:],
                                         mybir.ActivationFunctionType.Exp)
                else:
                    for w in range(2):
                        w0 = halves[w]
                        nc.scalar.activation(pT[:, w, w0:512],
                                             ps[:, w, w0:512],
                                             mybir.ActivationFunctionType.Exp)
                for w in range(2):
                    w0 = halves[w]
                    nc.vector.tensor_tensor(acc[:, w0:512], acc[:, w0:512],
                                            pT[:, w, w0:512],
                                            mybir.AluOpType.add)
                return pT, halves

            DEPTH = 3
            pts = [emit_scores(ii) for ii in range(min(DEPTH, nunit))]
            credit = 0
            for idx in range(nunit):
                pT, halves = pts[idx]
                if idx + DEPTH < nunit:
                    pts.append(emit_scores(idx + DEPTH))
                # fillers ride between the scores emission and the PV that
                # needs this unit's exp, maximizing the ACT slack
                credit += pull_num
                while credit >= pull_den:
                    tick(j, pull=True)
                    credit -= pull_den
                if credit > 0 and pull_num < pull_den:
                    tick(j, pull=False)
                u = order[idx]
                for w in range(2):
                    ki = 2 * u + w
                    w0 = halves[w]
                    nc.tensor.matmul(po[:, w0:512], v_sb[:, ki, :],
                                     pT[:, w, w0:512],
                                     start=(idx == 0 and w == 0),
                                     stop=(idx == nunit - 1 and w == 1))
            return po, acc

        def attn_fin(j, h, st, last):
            po, acc = st

            def run():
                # one normal-rate matmul: den[p,q] = sum_k acc[k,q] for all
                # p (the ones stationary broadcasts the partition sum)
                den = pden.tile([128, 512], F32, tag="big", name="den")
                nc.tensor.matmul(den[:], ones2[:], acc[:], start=True,
                                 stop=True)
                rinv = rinvp.tile([128, 512], F32, tag="rinv", name="rinv")
                nc.vector.reciprocal_approx_fast(rinv[:], den[:])
                att = smalls.tile([128, 512], BF16, tag="att", name="att")
                nc.vector.tensor_tensor(att[:], po[:], rinv[:],
                                        mybir.AluOpType.mult)
                nc.sync.dma_start(ag_ins[j][ts(h, 128), :], att[:])
                if last:
                    nc.gpsimd.collective_compute(
                        "AllGather", mybir.AluOpType.bypass,
                        replica_groups=[list(range(N_CORES))],
                        ins=[ag_ins[j].opt()], outs=[ag_outs[j].opt()],
                    )
            return run

        # ================= schedule =================
        # filler phases: chunk-s attention pulls proj(s+1); chunk-3 pulls
        # o_proj(0)/(1); the tail drains o_proj(2)/(3).
        for s in (1, 2, 3):
            phase_q[s - 1].append(load_x(s))
            phase_q[s - 1].extend(gen_proj_v(s, ppj))
            phase_q[s - 1].extend(gen_proj_k(s, ppj))
            for h in range(HEADS_PER_CORE):
                phase_q[s - 1].extend(gen_proj_q(s, h, ppj))
        phase_q[2].append(load_wo())
        for s in (0, 1):
            phase_q[3].append(load_a(s))
            for ft in range(4):
                phase_q[3].extend(gen_oproj(s, ft, ppj))
        for s in (2, 3):
            phase_q[4].append(load_a(s))
            for ft in range(4):
                phase_q[4].extend(gen_oproj(s, ft, psb))

        # chunk-0: emit only V, K, q0 up front (spread over three PSUM
        # pools so the rope chains never serialize the accumulator bank);
        # q1..q3 ride as fillers inside heads 0-2 (pull 4/unit covers one
        # q group per head).
        for q in gen_proj_v(0, psb):
            q()
        for q in gen_proj_k(0, psb):
            q()
        for q in gen_proj_q(0, 0, ppj):
            q()
        c0 = deque()
        c0.extend(gen_proj_q(0, 1, pden))
        c0.extend(gen_proj_q(0, 2, psb))
        c0.extend(gen_proj_q(0, 3, psb))
        c0.extend(phase_q[0])
        phase_q[0] = c0
        tick(0)
        tick(0)

        # per-chunk (pull_num, pull_den): c0 covers its own q groups, c1/c2
        # absorb most of the next chunk's proj so the drain shrinks, c3
        # paces o_proj; the last head minimizes pulls so the final
        # AllGather triggers ASAP.
        rates = {0: (4, 1), 1: (2, 1), 2: (2, 1), 3: (1, 1)}
        for s in range(SC):
            for h in range(HEADS_PER_CORE):
                # emission-order guard: this head's q rope (and the chunk's
                # k/v epilogues) must be EMITTED before its scores/PV are,
                # else they'd read stale SBUF (no forward deps exist)
                while not (ready.get(("q", s, h)) and ready.get(("k", s))
                           and ready.get(("v", s))):
                    if not tick(s):
                        raise RuntimeError(f"filler underflow at {s},{h}")
                if s == SC - 1 and h == HEADS_PER_CORE - 1:
                    pn, pd = 1, 3
                else:
                    pn, pd = rates[s]
                st = attn_head(s, h, pn, pd)
                fins.append([2, attn_fin(s, h, st,
                                         last=(h == HEADS_PER_CORE - 1))])
            if s < SC - 1:
                # drain most of this chunk's proj fillers (next chunk's
                # scores need qT/kT/v soon) but keep a tail of quanta to
                # absorb the rope-boundary stalls inside the next heads
                while len(phase_q[s]) > 12:
                    tick(s)

        # ---- tail: flush remaining fins + o_proj quanta
        while fins or any(phase_q):
            if not any(phase_q):
                for item in fins:
                    item[0] = 0
            tick(SC - 1)

    nc.finalize()
    return nc


def _prep_inputs(hidden_states, Wq, Wk, Wv, Wo, position_ids):
    """Slice/cast per-core inputs (host-side layout prep only)."""
    bf = ml_dtypes.bfloat16
    x = np.ascontiguousarray(np.asarray(hidden_states, np.float32)[0].T).astype(bf)
    scale = 1.0 / np.sqrt(HEAD_DIM)
    # rotary tables, [head_dim, seq]; sin signed (first half negated)
    invf_half = (1.0 / (ROPE_THETA ** (np.arange(0, HEAD_DIM, 2, dtype=np.float64)
                                       / HEAD_DIM)))
    invf = np.concatenate([invf_half, invf_half])  # [128]
    pos = np.asarray(position_ids, np.float64).reshape(S)
    ang = invf[:, None] * pos[None, :]             # [128, S]
    cosT = np.cos(ang).astype(np.float32)
    sinT = np.sin(ang).astype(np.float32)
    sinT[:HEAD_DIM // 2] *= -1.0
    in_maps = []
    for c in range(N_CORES):
        wq_c = (np.asarray(Wq, np.float32)[c * QSLICE:(c + 1) * QSLICE] * scale)
        wk_c = np.asarray(Wk, np.float32)[c * HEAD_DIM:(c + 1) * HEAD_DIM]
        wv_c = np.asarray(Wv, np.float32)[c * HEAD_DIM:(c + 1) * HEAD_DIM]
        wkv_c = np.concatenate([wk_c, wv_c], axis=0)   # [256, 4096]
        wo_c = np.asarray(Wo, np.float32)[c * QSLICE:(c + 1) * QSLICE]
        in_maps.append({
            "xT": x,
            "wqT": np.ascontiguousarray(wq_c.T).astype(bf),
            "wkvT": np.ascontiguousarray(wkv_c.T).astype(bf),
            "woT": np.ascontiguousarray(wo_c.T).astype(bf),
            "cosT": cosT,
            "sinT": sinT,
        })
    return in_maps


def kernel(hidden_states, Wq, Wk, Wv, Wo, position_ids):
    from concourse.bass_utils import run_bass_kernel_spmd
    if "nc" not in _cache:
        _cache["nc"] = build_nc()
    nc = _cache["nc"]
    in_maps = _prep_inputs(hidden_states, Wq, Wk, Wv, Wo, position_ids)
    res = run_bass_kernel_spmd(nc, in_maps, core_ids=list(range(N_CORES)))
    out = np.concatenate([res.results[c]["outT"].T for c in range(N_CORES)], axis=1)
    return out[None].astype(np.float32)


# revision 20
# speedup vs baseline: 1.0415x; 1.0318x over previous
"""Llama GQA attention (B=1, S=2048, H=4096, 32 heads / 8 KV heads, RoPE, causal)
as a tensor-parallel Bass/Tile kernel on 8 Trainium2 NeuronCores.

Sharding: core c computes Q heads [4c, 4c+4) and KV head c (GQA groups align),
full causal attention for those heads, then AllGathers the (transposed)
attention outputs and computes output features [512c, 512c+512) of o_proj.

v9 (bf16):
- Fine-grained filler weave: all proj / o_proj work is decomposed into ~1us
  "quanta" (4 k-tile matmul steps) held in per-chunk deques. The attention
  unit loop pulls one quantum after every score/PV unit, so the PE always
  has independent matmul work while the Scalar engine's EXP stream (the
  local pacer, ~1.3ns/elem) catches up. Chunk-3 attention pulls o_proj(0)/
  o_proj(1) quanta (their AllGathers complete long before), so the PE never
  idles long enough for the HAM clock gate to re-throttle.
- Softmax denominator: ONE normal-rate matmul with a ones[128,128]
  stationary computes the partition sum broadcast to all 128 partitions
  (v7 used a ones[128,1] + row-broadcast pair, each streaming at the slow
  1.2GHz narrow-output path), then reciprocal + po*rinv on DVE.
- Critical input DMAs are emitted before any const-pool memset so the DMA
  queues start moving at ~6us instead of ~11.6us (gpsimd preamble work was
  gating the first transfers).
- Proj groups are single-PSUM-bank ([128,512]); chunk-0 runs V,K,q0 up
  front across three pools, then starts attention with q1..q3 as fillers
  (pull 4/unit), with emission-order guards ensuring each head's rope is
  emitted before its scores (no forward deps exist for stale reads).
- PSUM: scores 2x[128,2,512] (also chunk-0 proj + tail o_proj, for double
  buffering), filler proj/o_proj 1x[128,512], denominator/V-transpose
  1x bank, po 2x[128,512] = exactly 8 banks.
- Finishers ride 2 units into the next head's stream; the AllGather for a
  chunk triggers right after its last att DMA (gpsimd carries only AG
  triggers); exp batched 2 k-blocks per ACT op; DEPTH=3 score pipeline.
"""
import numpy as np
import ml_dtypes
from collections import deque
from contextlib import ExitStack

import concourse.bass as bass
import concourse.mybir as mybir
import concourse.tile as tile
from concourse import bacc
from concourse.bass import ts, ds
from concourse.masks import make_identity

N_CORES = 8
S = 2048
HIDDEN = 4096
NUM_HEADS = 32
HEAD_DIM = 128
HEADS_PER_CORE = NUM_HEADS // N_CORES          # 4
QSLICE = HEADS_PER_CORE * HEAD_DIM             # 512
KT = HIDDEN // 128                             # 32 contraction tiles
SC = S // 512                                  # 4 seq chunks of 512
ROPE_THETA = 10000.0

F32 = mybir.dt.float32
BF16 = mybir.dt.bfloat16

_cache = {}


def build_nc():
    nc = bacc.Bacc("TRN2", target_bir_lowering=False, debug=False,
                   num_devices=N_CORES)
    xT = nc.dram_tensor("xT", [HIDDEN, S], BF16, kind="ExternalInput").ap()
    wqT = nc.dram_tensor("wqT", [HIDDEN, QSLICE], BF16, kind="ExternalInput").ap()
    wkvT = nc.dram_tensor("wkvT", [HIDDEN, 2 * HEAD_DIM], BF16,
                          kind="ExternalInput").ap()
    woT = nc.dram_tensor("woT", [HIDDEN, QSLICE], BF16, kind="ExternalInput").ap()
    cosT = nc.dram_tensor("cosT", [HEAD_DIM, S], F32, kind="ExternalInput").ap()
    sinT = nc.dram_tensor("sinT", [HEAD_DIM, S], F32, kind="ExternalInput").ap()
    outT = nc.dram_tensor("outT", [QSLICE, S], F32, kind="ExternalOutput").ap()

    xT_r = xT.rearrange("(kt p) s -> p kt s", p=128)
    wqT_r = wqT.rearrange("(kt p) m -> p kt m", p=128)
    wkvT_r = wkvT.rearrange("(kt p) m -> p kt m", p=128)
    woT_r = woT.rearrange("(kt p) m -> p kt m", p=128)

    with tile.TileContext(nc) as tc, ExitStack() as ctx:
        const = ctx.enter_context(tc.tile_pool(name="const", bufs=1))
        bigw = ctx.enter_context(tc.tile_pool(name="bigw", bufs=1))
        slab = ctx.enter_context(tc.tile_pool(name="slab", bufs=2))
        f32t = ctx.enter_context(tc.tile_pool(name="f32t", bufs=4))
        ppool = ctx.enter_context(tc.tile_pool(name="ppool", bufs=5))
        accp = ctx.enter_context(tc.tile_pool(name="accp", bufs=2))
        smalls = ctx.enter_context(tc.tile_pool(name="smalls", bufs=2))
        rinvp = ctx.enter_context(tc.tile_pool(name="rinvp", bufs=2))
        otp = ctx.enter_context(tc.tile_pool(name="otp", bufs=4))
        dram = ctx.enter_context(tc.tile_pool(name="dram", bufs=1, space="DRAM"))
        # PSUM banks: psb 2x[128,2,512]=4, ppj 1x[128,512]=1,
        #             pden 1x(den|ptr)=1, ppo 2x[128,512]=2  -> 8 total
        psb = ctx.enter_context(tc.tile_pool(name="psb", bufs=2, space="PSUM"))
        ppj = ctx.enter_context(tc.tile_pool(name="ppj", bufs=1, space="PSUM"))
        pden = ctx.enter_context(tc.tile_pool(name="pden", bufs=1, space="PSUM"))
        ppo = ctx.enter_context(tc.tile_pool(name="ppo", bufs=2, space="PSUM"))

        # ---- persistent tiles (allocation only; no instructions yet)
        ones2 = const.tile([128, 128], BF16, name="ones2")
        ident = const.tile([128, 128], BF16, name="ident")
        maskb = const.tile([128, 4, 512], BF16, name="maskb")
        cos_sb = const.tile([128, S], F32, name="cos_sb")
        sin_sb = const.tile([128, S], F32, name="sin_sb")
        qT_sb = const.tile([128, HEADS_PER_CORE, S], BF16, name="qT_sb")
        kT_sb = const.tile([128, S], BF16, name="kT_sb")
        v_sb = const.tile([128, S // 128, HEAD_DIM], BF16, name="v_sb")
        wkv_sb = const.tile([128, KT, 2 * HEAD_DIM], BF16, name="wkv_sb")
        # wq and wo share one 32KB/part slot; wo loads once proj is done
        wq_sb = bigw.tile([128, KT, QSLICE], BF16, tag="bigw", name="wq_sb")

        ag_ins = [dram.tile([QSLICE, 512], BF16, tag=f"agin{j}",
                            name=f"agin{j}") for j in range(SC)]
        ag_outs = [dram.tile([NUM_HEADS * HEAD_DIM, 512], BF16,
                             addr_space="Shared", tag=f"agout{j}",
                             name=f"agout{j}") for j in range(SC)]

        # ---- critical DMAs FIRST (before any const memset, so the first
        # transfers aren't queued behind gpsimd preamble work)
        x_slab0 = slab.tile([128, KT, 512], BF16, tag="slab", name="x_slab0")
        for g in range(4):  # fine-grained first quarter for a fast ramp
            kts = ds(g, 1)
            nc.sync.dma_start(x_slab0[:, kts, :], xT_r[:, kts, 0:512])
            nc.sync.dma_start(wkv_sb[:, kts, :], wkvT_r[:, kts, :])
        for g in range(1, 8):
            kts = ds(4 * g, 4)
            nc.sync.dma_start(x_slab0[:, kts, :], xT_r[:, kts, 0:512])
            nc.sync.dma_start(wkv_sb[:, kts, :], wkvT_r[:, kts, :])
        nc.sync.dma_start(cos_sb[:], cosT[:])
        nc.sync.dma_start(sin_sb[:], sinT[:])
        for g in range(8):
            kts = ds(4 * g, 4)
            nc.sync.dma_start(wq_sb[:, kts, :], wqT_r[:, kts, :])

        # ---- constants
        nc.vector.memset(ones2[:], 1.0)
        make_identity(nc, ident[:])
        # causal mask bias tiles: maskb[k,d,q] = 0 where q-128d-k >= 0 else -1e9
        nc.gpsimd.memset(maskb[:], 0.0)
        for d in range(4):
            nc.gpsimd.affine_select(
                maskb[:, d, :], maskb[:, d, :], pattern=[[1, 512]],
                compare_op=mybir.AluOpType.is_ge, fill=-1.0e9,
                base=-128 * d, channel_multiplier=-1)

        def rope(dst, src, s):
            """dst = src*cos + rotate_half(src)*sin_signed for seq chunk s."""
            rot = f32t.tile([128, 512], F32, tag="f32t", name="rot")
            nc.vector.tensor_tensor(rot[0:64, :], src[64:128, :],
                                    sin_sb[0:64, ts(s, 512)],
                                    mybir.AluOpType.mult)
            nc.vector.tensor_tensor(rot[64:128, :], src[0:64, :],
                                    sin_sb[64:128, ts(s, 512)],
                                    mybir.AluOpType.mult)
            cq = f32t.tile([128, 512], F32, tag="f32t", name="cq")
            nc.vector.tensor_tensor(cq[:], src[:], cos_sb[:, ts(s, 512)],
                                    mybir.AluOpType.mult)
            nc.vector.tensor_tensor(dst, cq[:], rot[:], mybir.AluOpType.add)

        x_slabs = {0: x_slab0}
        wo_holder = {}
        a_slabs = {}
        ready = {}  # emission-order guards: set when a proj epilogue emits

        # ================= filler quantum machinery =================
        # Each quantum is a zero-arg closure emitting ~1us of PE work (plus
        # any non-PE epilogue). Finishers are queued with a tick delay so
        # their producer chains (DVE acc) settle inside the next stream.
        fins = []                                   # [delay_ticks, fn]
        phase_q = [deque() for _ in range(SC + 1)]  # per-chunk filler deques

        def pull_filler(s):
            for d in phase_q:
                if d:
                    d.popleft()()
                    return True
            return False

        def tick(s, pull=True):
            due = []
            for item in fins:
                item[0] -= 1
            while fins and fins[0][0] <= 0:
                due.append(fins.pop(0)[1])
            for f in due:
                f()
            if not due and pull:
                return pull_filler(s)
            return True

        # ---- single-bank proj group generators (32 MMs + epilogue each)
        def gen_group(pool, mm_of_kt, epi):
            box = {}

            def begin():
                box["p"] = pool.tile([128, 512], F32, tag="big", name="pacc")
                step(0)()

            def step(k0):
                def run():
                    for kt in range(k0, k0 + 4):
                        nc.tensor.matmul(box["p"][:], *mm_of_kt(kt),
                                         start=(kt == 0), stop=(kt == KT - 1))
                return run

            def end():
                epi(box["p"])

            return [begin] + [step(k) for k in range(4, KT, 4)] + [end]

        def gen_proj_v(s, pool):
            def epi(p):
                vt = smalls.tile([128, 512], BF16, tag="vt", name="vt")
                nc.scalar.copy(vt[:], p[:])
                ptr = pden.tile([128, 4, 128], BF16, tag="big", name="ptr")
                for t in range(4):
                    nc.tensor.transpose(ptr[:, t, :], vt[:, ts(t, 128)],
                                        ident[:])
                nc.scalar.copy(v_sb[:, ds(4 * s, 4), :], ptr[:])
                ready[("v", s)] = True
            return gen_group(
                pool,
                lambda kt: (wkv_sb[:, kt, ds(128, 128)], x_slabs[s][:, kt, :]),
                epi)

        def gen_proj_k(s, pool):
            # rope reads the PSUM accumulator directly (cross-partition
            # tensor_tensor needs a PSUM input; SBUF+SBUF must align)
            def epi(p):
                rope(kT_sb[:, ts(s, 512)], p[:], s)
                ready[("k", s)] = True
            return gen_group(
                pool,
                lambda kt: (wkv_sb[:, kt, ds(0, 128)], x_slabs[s][:, kt, :]),
                epi)

        def gen_proj_q(s, h, pool):
            def epi(p):
                rope(qT_sb[:, h, ts(s, 512)], p[:], s)
                ready[("q", s, h)] = True
            return gen_group(
                pool,
                lambda kt: (wq_sb[:, kt, ts(h, 128)], x_slabs[s][:, kt, :]),
                epi)

        def gen_oproj(s, ft, pool):
            def epi(p):
                ot = otp.tile([128, 512], F32, tag="ot", name="ot")
                nc.scalar.copy(ot[:], p[:])
                nc.sync.dma_start(outT[ds(ft * 128, 128), ts(s, 512)], ot[:])
            return gen_group(
                pool,
                lambda kt: (wo_holder[0][:, kt, ts(ft, 128)],
                            a_slabs[s][:, kt, :]),
                epi)

        def load_x(s):
            def run():
                xs = slab.tile([128, KT, 512], BF16, tag="slab",
                               name="x_slab")
                nc.sync.dma_start(xs[:], xT_r[:, :, ts(s, 512)])
                x_slabs[s] = xs
            return run

        def load_wo():
            def run():
                wo = bigw.tile([128, KT, QSLICE], BF16, tag="bigw",
                               name="wo_sb")
                for f in range(4):
                    nc.sync.dma_start(wo[:, :, ts(f, 128)],
                                      woT_r[:, :, ts(f, 128)])
                wo_holder[0] = wo
            return run

        def load_a(s):
            def run():
                a_slab = slab.tile([128, KT, 512], BF16, tag="slab",
                                   name="a_slab")
                ag_r = ag_outs[s].rearrange("(kt p) s -> p kt s", p=128)
                for g in range(4):
                    kts = ds(8 * g, 8)
                    nc.sync.dma_start(a_slab[:, kts, :], ag_r[:, kts, :])
                a_slabs[s] = a_slab
            return run

        # ================= attention =================
        def attn_head(j, h, pull_num, pull_den):
            """Scores + exp + mask + denom-acc + PV, DEPTH=3 pipeline.
            Pulls pull_num filler quanta every pull_den units."""
            nunit = 2 * (j + 1)
            po = ppo.tile([128, 512], F32, tag="po", name="po")
            acc = accp.tile([128, 512], BF16, tag="acc", name="acc")
            nc.vector.memset(acc[:], 0.0)
            # diagonal (masked) units first; their scores/exp/PV shrink to
            # the causally-valid rectangle q >= 128d
            order = list(range(nunit))[::-1]

            def emit_scores(idx):
                u = order[idx]
                ps = psb.tile([128, 2, 512], F32, tag="big", name="ps")
                halves = []
                for w in range(2):
                    ki = 2 * u + w
                    d = ki - 4 * j
                    w0 = 128 * d if d > 0 else 0
                    halves.append(w0)
                    nc.tensor.matmul(ps[:, w, w0:512],
                                     kT_sb[:, ts(ki, 128)],
                                     qT_sb[:, h, ds(512 * j + w0, 512 - w0)],
                                     start=True, stop=(d < 0))
                    if d >= 0:  # diagonal block: add -1e9 causal bias on PE
                        nc.tensor.matmul(ps[:, w, w0:512], ident[:],
                                         maskb[:, d, w0:512],
                                         start=False, stop=True)
                pT = ppool.tile([128, 2, 512], BF16, tag="pT", name="pT")
                if halves[0] == 0 and halves[1] == 0:
                    nc.scalar.activation(pT[:], ps[:],
                                         mybir.ActivationFunctionType.Exp)
                else:
                    for w in range(2):
                        w0 = halves[w]
                        nc.scalar.activation(pT[:, w, w0:512],
                                             ps[:, w, w0:512],
                                             mybir.ActivationFunctionType.Exp)
                for w in range(2):
                    w0 = halves[w]
                    nc.vector.tensor_tensor(acc[:, w0:512], acc[:, w0:512],
                                            pT[:, w, w0:512],
                                            mybir.AluOpType.add)
                return pT, halves

            DEPTH = 3
            pts = [emit_scores(ii) for ii in range(min(DEPTH, nunit))]
            credit = 0
            for idx in range(nunit):
                pT, halves = pts[idx]
                if idx + DEPTH < nunit:
                    pts.append(emit_scores(idx + DEPTH))
                # fillers ride between the scores emission and the PV that
                # needs this unit's exp, maximizing the ACT slack
                credit += pull_num
                while credit >= pull_den:
                    tick(j, pull=True)
                    credit -= pull_den
                if credit > 0 and pull_num < pull_den:
                    tick(j, pull=False)
                u = order[idx]
                for w in range(2):
                    ki = 2 * u + w
                    w0 = halves[w]
                    nc.tensor.matmul(po[:, w0:512], v_sb[:, ki, :],
                                     pT[:, w, w0:512],
                                     start=(idx == 0 and w == 0),
                                     stop=(idx == nunit - 1 and w == 1))
            return po, acc

        def attn_fin(j, h, st, last):
            po, acc = st

            def run():
                # one normal-rate matmul: den[p,q] = sum_k acc[k,q] for all
                # p (the ones stationary broadcasts the partition sum)
                den = pden.tile([128, 512], F32, tag="big", name="den")
                nc.tensor.matmul(den[:], ones2[:], acc[:], start=True,
                                 stop=True)
                rinv = rinvp.tile([128, 512], F32, tag="rinv", name="rinv")
                nc.vector.reciprocal_approx_fast(rinv[:], den[:])
                att = smalls.tile([128, 512], BF16, tag="att", name="att")
                nc.vector.tensor_tensor(att[:], po[:], rinv[:],
                                        mybir.AluOpType.mult)
                nc.sync.dma_start(ag_ins[j][ts(h, 128), :], att[:])
                if last:
                    nc.gpsimd.collective_compute(
                        "AllGather", mybir.AluOpType.bypass,
                        replica_groups=[list(range(N_CORES))],
                        ins=[ag_ins[j].opt()], outs=[ag_outs[j].opt()],
                    )
            return run

        # ================= schedule =================
        # filler phases: chunk-s attention pulls proj(s+1); chunk-3 pulls
        # o_proj(0)/(1); the tail drains o_proj(2)/(3).
        for s in (1, 2, 3):
            phase_q[s - 1].append(load_x(s))
            phase_q[s - 1].extend(gen_proj_v(s, ppj))
            phase_q[s - 1].extend(gen_proj_k(s, ppj))
            for h in range(HEADS_PER_CORE):
                # alternate banks so adjacent q-groups never serialize on
                # one PSUM bank held by the predecessor's rope reads (the
                # chunk-0 q1-in-pden pattern; pden is fins-only again by
                # chunk 3, so AllGather triggers are unaffected)
                phase_q[s - 1].extend(gen_proj_q(s, h,
                                                 pden if h % 2 else ppj))
        phase_q[2].append(load_wo())
        for s in (0, 1):
            phase_q[3].append(load_a(s))
            for ft in range(4):
                phase_q[3].extend(gen_oproj(s, ft, ppj))
        for s in (2, 3):
            phase_q[4].append(load_a(s))
            for ft in range(4):
                phase_q[4].extend(gen_oproj(s, ft, psb))

        # chunk-0: emit only V, K, q0 up front (spread over three PSUM
        # pools so the rope chains never serialize the accumulator bank);
        # q1..q3 ride as fillers inside heads 0-2 (pull 4/unit covers one
        # q group per head).
        for q in gen_proj_v(0, psb):
            q()
        for q in gen_proj_k(0, psb):
            q()
        for q in gen_proj_q(0, 0, ppj):
            q()
        c0 = deque()
        c0.extend(gen_proj_q(0, 1, pden))
        c0.extend(gen_proj_q(0, 2, psb))
        c0.extend(gen_proj_q(0, 3, psb))
        c0.extend(phase_q[0])
        phase_q[0] = c0
        tick(0)
        tick(0)

        # per-chunk (pull_num, pull_den): c0 covers its own q groups, c1/c2
        # absorb most of the next chunk's proj so the drain shrinks, c3
        # paces o_proj; the last head minimizes pulls so the final
        # AllGather triggers ASAP.
        rates = {0: (4, 1), 1: (2, 1), 2: (2, 1), 3: (1, 1)}
        for s in range(SC):
            for h in range(HEADS_PER_CORE):
                # emission-order guard: this head's q rope (and the chunk's
                # k/v epilogues) must be EMITTED before its scores/PV are,
                # else they'd read stale SBUF (no forward deps exist)
                while not (ready.get(("q", s, h)) and ready.get(("k", s))
                           and ready.get(("v", s))):
                    if not tick(s):
                        raise RuntimeError(f"filler underflow at {s},{h}")
                if s == SC - 1 and h == HEADS_PER_CORE - 1:
                    pn, pd = 1, 3
                else:
                    pn, pd = rates[s]
                st = attn_head(s, h, pn, pd)
                fins.append([2, attn_fin(s, h, st,
                                         last=(h == HEADS_PER_CORE - 1))])
            if s < SC - 1:
                # drain most of this chunk's proj fillers (next chunk's
                # scores need qT/kT/v soon) but keep a tail of quanta to
                # absorb the rope-boundary stalls inside the next heads
                while len(phase_q[s]) > 12:
                    tick(s)

        # ---- tail: flush remaining fins + o_proj quanta
        while fins or any(phase_q):
            if not any(phase_q):
                for item in fins:
                    item[0] = 0
            tick(SC - 1)

    nc.finalize()
    return nc


def _prep_inputs(hidden_states, Wq, Wk, Wv, Wo, position_ids):
    """Slice/cast per-core inputs (host-side layout prep only)."""
    bf = ml_dtypes.bfloat16
    x = np.ascontiguousarray(np.asarray(hidden_states, np.float32)[0].T).astype(bf)
    scale = 1.0 / np.sqrt(HEAD_DIM)
    # rotary tables, [head_dim, seq]; sin signed (first half negated)
    invf_half = (1.0 / (ROPE_THETA ** (np.arange(0, HEAD_DIM, 2, dtype=np.float64)
                                       / HEAD_DIM)))
    invf = np.concatenate([invf_half, invf_half])  # [128]
    pos = np.asarray(position_ids, np.float64).reshape(S)
    ang = invf[:, None] * pos[None, :]             # [128, S]
    cosT = np.cos(ang).astype(np.float32)
    sinT = np.sin(ang).astype(np.float32)
    sinT[:HEAD_DIM // 2] *= -1.0
    in_maps = []
    for c in range(N_CORES):
        wq_c = (np.asarray(Wq, np.float32)[c * QSLICE:(c + 1) * QSLICE] * scale)
        wk_c = np.asarray(Wk, np.float32)[c * HEAD_DIM:(c + 1) * HEAD_DIM]
        wv_c = np.asarray(Wv, np.float32)[c * HEAD_DIM:(c + 1) * HEAD_DIM]
        wkv_c = np.concatenate([wk_c, wv_c], axis=0)   # [256, 4096]
        wo_c = np.asarray(Wo, np.float32)[c * QSLICE:(c + 1) * QSLICE]
        in_maps.append({
            "xT": x,
            "wqT": np.ascontiguousarray(wq_c.T).astype(bf),
            "wkvT": np.ascontiguousarray(wkv_c.T).astype(bf),
            "woT": np.ascontiguousarray(wo_c.T).astype(bf),
            "cosT": cosT,
            "sinT": sinT,
        })
    return in_maps


def kernel(hidden_states, Wq, Wk, Wv, Wo, position_ids):
    from concourse.bass_utils import run_bass_kernel_spmd
    if "nc" not in _cache:
        _cache["nc"] = build_nc()
    nc = _cache["nc"]
    in_maps = _prep_inputs(hidden_states, Wq, Wk, Wv, Wo, position_ids)
    res = run_bass_kernel_spmd(nc, in_maps, core_ids=list(range(N_CORES)))
    out = np.concatenate([res.results[c]["outT"].T for c in range(N_CORES)], axis=1)
    return out[None].astype(np.float32)
